# revision 3
# baseline (speedup 1.0000x reference)
"""AttentionHyperNet kernel — data-parallel across 8 NeuronCores.

Wire-optimized path: the tunnel to the device pod is the bottleneck
(~80 MB/s up, ~50 MB/s down, ~90 ms per RPC), so the kernel
  * drops masked entity rows on the host (they cannot affect the
    output: masked agents are zeroed, masked entities get -inf
    attention logits) and ships only valid rows as float16,
  * packs entities + params + per-sample counts/offsets/output
    indices into ONE uint16 buffer per core -> a single sharded
    device_put,
  * computes in fp32 on device from the fp16 payload,
  * returns only the valid agent rows as fp16 and scatters them into
    the full fp32 (4096, 64, 32) output on the host.

Self-contained: no sibling imports, shapes hardcoded.
"""

import os
import sys
import time

import numpy as np

N_AGENTS = 64
N_HEADS = 4
N_CORES = 8
BS = 4096
NE = 128
FD = 19
E = 128
M = 32
SH = BS // N_CORES
HD = E // N_HEADS

N_PAR = FD * E + E + E * 3 * E + E * E + E + E * M + M

_DEBUG = bool(os.environ.get("BASSKERNEL_DEBUG"))


def _dbg(msg):
    if _DEBUG:
        print(f"[kernel] {msg}", file=sys.stderr, flush=True)


def _round_up(x, m):
    return ((int(x) + m - 1) // m) * m


_JAX_STATE = {}
_FWD_CACHE = {}


def _jax_state():
    if _JAX_STATE:
        return _JAX_STATE
    import jax
    from jax.sharding import Mesh, NamedSharding, PartitionSpec as P

    devs = jax.devices()[:N_CORES]
    if len(devs) < N_CORES:
        raise RuntimeError("need 8 cores")
    mesh = Mesh(np.array(devs), ("b",))
    _JAX_STATE["jax"] = jax
    _JAX_STATE["mesh"] = mesh
    _JAX_STATE["shard"] = NamedSharding(mesh, P("b"))
    _JAX_STATE["P"] = P
    return _JAX_STATE


def _get_fwd(cap_e, cap_a, ne_pad, na_pad):
    key = (cap_e, cap_a, ne_pad, na_pad)
    fn = _FWD_CACHE.get(key)
    if fn is not None:
        return fn
    st = _jax_state()
    jax = st["jax"]
    mesh = st["mesh"]
    P = st["P"]
    import jax.numpy as jnp
    from jax.experimental.shard_map import shard_map

    n_ent = cap_e * FD
    o_par = n_ent
    o_cnt = o_par + N_PAR
    o_off = o_cnt + SH
    o_idx = o_off + SH

    def core_fwd(buf):  # (1, total) uint16 on one core
        buf = buf.reshape(-1)
        ent = jax.lax.bitcast_convert_type(
            buf[:n_ent].reshape(cap_e, FD), jnp.float16
        ).astype(jnp.float32)
        par = jax.lax.bitcast_convert_type(
            buf[o_par : o_par + N_PAR], jnp.float16
        ).astype(jnp.float32)
        pos = [0]

        def take(n, shape):
            v = par[pos[0] : pos[0] + n].reshape(shape)
            pos[0] += n
            return v

        W1 = take(FD * E, (FD, E))
        b1 = take(E, (E,))
        Wqkv = take(E * 3 * E, (E, 3 * E))
        Wout = take(E * E, (E, E))
        bout = take(E, (E,))
        W2 = take(E * M, (E, M))
        b2 = take(M, (M,))
        cnt = buf[o_cnt : o_cnt + SH].astype(jnp.int32)
        off = buf[o_off : o_off + SH].astype(jnp.int32)
        oidx = buf[o_idx : o_idx + cap_a].astype(jnp.int32)

        gidx = jnp.clip(
            off[:, None] + jnp.arange(ne_pad)[None, :], 0, cap_e - 1
        )
        pe = ent[gidx]  # (SH, ne_pad, FD)
        x1 = jax.nn.relu(pe @ W1 + b1)
        qkv = x1 @ Wqkv  # (SH, ne_pad, 3E)
        q = qkv[:, :na_pad, :E]
        k = qkv[:, :, E : 2 * E]
        v = qkv[:, :, 2 * E :]
        qh = q.reshape(SH, na_pad, N_HEADS, HD)
        kh = k.reshape(SH, ne_pad, N_HEADS, HD)
        vh = v.reshape(SH, ne_pad, N_HEADS, HD)
        logits = jnp.einsum("sqhd,skhd->shqk", qh, kh) * (
            1.0 / np.sqrt(float(HD))
        )
        kmask = jnp.arange(ne_pad)[None, :] < cnt[:, None]
        logits = jnp.where(kmask[:, None, None, :], logits, -1e30)
        w = jax.nn.softmax(logits, axis=-1)
        attn = jnp.einsum("shqk,skhd->sqhd", w, vh).reshape(SH, na_pad, E)
        x2 = attn @ Wout + bout
        x3 = x2 @ W2 + b2
        out = x3.reshape(SH * na_pad, M)[oidx]
        return out.astype(jnp.float16)[None]

    fwd = jax.jit(
        shard_map(
            core_fwd, mesh=mesh, in_specs=P("b"), out_specs=P("b"),
            check_rep=False,
        )
    )
    _FWD_CACHE[key] = fwd
    return fwd


def _run_packed(entities, entity_mask, W1, b1, Wqkv, Wout, bout, W2, b2):
    st = _jax_state()
    jax = st["jax"]
    shard = st["shard"]

    t0 = time.perf_counter()
    ent = np.ascontiguousarray(entities, np.float32).reshape(BS, NE, FD)
    valid = np.ascontiguousarray(entity_mask).reshape(BS, NE) == 0
    cnt_e = valid.sum(1).astype(np.int64)
    va = valid[:, :N_AGENTS]
    cnt_a = va.sum(1).astype(np.int64)
    ce = cnt_e.reshape(N_CORES, SH)
    ca = cnt_a.reshape(N_CORES, SH)
    core_tot_e = ce.sum(1)
    core_tot_a = ca.sum(1)

    ne_pad = max(8, _round_up(ce.max(), 8))
    na_pad = max(8, _round_up(ca.max(), 8))
    cap_e = max(2048, _round_up(core_tot_e.max(), 2048))
    cap_a = max(1024, _round_up(core_tot_a.max(), 1024))

    n_ent = cap_e * FD
    o_par = n_ent
    o_cnt = o_par + N_PAR
    o_off = o_cnt + SH
    o_idx = o_off + SH
    total = o_idx + cap_a

    f16 = ent[valid].astype(np.float16)  # (TOT, FD), sample-major order
    cum = np.zeros(BS + 1, np.int64)
    np.cumsum(cnt_e, out=cum[1:])
    params16 = (
        np.concatenate(
            [
                np.asarray(W1, np.float32).ravel(),
                np.asarray(b1, np.float32).ravel(),
                np.asarray(Wqkv, np.float32).ravel(),
                np.asarray(Wout, np.float32).ravel(),
                np.asarray(bout, np.float32).ravel(),
                np.asarray(W2, np.float32).ravel(),
                np.asarray(b2, np.float32).ravel(),
            ]
        )
        .astype(np.float16)
        .view(np.uint16)
    )

    buf = np.zeros((N_CORES, total), np.uint16)
    for c in range(N_CORES):
        r0 = cum[c * SH]
        r1 = cum[(c + 1) * SH]
        n = int(r1 - r0)
        if n:
            buf[c, : n * FD] = f16[r0:r1].reshape(-1).view(np.uint16)
        buf[c, o_par : o_par + N_PAR] = params16
        buf[c, o_cnt : o_cnt + SH] = ce[c].astype(np.uint16)
        buf[c, o_off : o_off + SH] = (cum[c * SH : (c + 1) * SH] - r0).astype(
            np.uint16
        )
        ta = int(core_tot_a[c])
        if ta:
            i_ids = np.repeat(np.arange(SH), ca[c])
            cum_a = np.zeros(SH + 1, np.int64)
            np.cumsum(ca[c], out=cum_a[1:])
            j_ids = np.arange(ta) - np.repeat(cum_a[:-1], ca[c])
            buf[c, o_idx : o_idx + ta] = (i_ids * na_pad + j_ids).astype(
                np.uint16
            )
    t1 = time.perf_counter()

    g = jax.device_put(buf, shard)
    g.block_until_ready()
    t2 = time.perf_counter()

    fwd = _get_fwd(cap_e, cap_a, ne_pad, na_pad)
    out = fwd(g)
    out.block_until_ready()
    t3 = time.perf_counter()

    shards = sorted(
        out.addressable_shards, key=lambda s: s.index[0].start or 0
    )
    datas = [s.data for s in shards]
    for d in datas:
        d.copy_to_host_async()
    vals = [np.asarray(d) for d in datas]
    t4 = time.perf_counter()

    res = np.zeros((BS, N_AGENTS, M), np.float32)
    picked = np.concatenate(
        [v.reshape(cap_a, M)[: int(core_tot_a[c])] for c, v in enumerate(vals)]
    )
    res[va] = picked.astype(np.float32)
    t5 = time.perf_counter()
    _dbg(
        f"pack:{t1 - t0:.3f} up:{t2 - t1:.3f} compute:{t3 - t2:.3f} "
        f"fetch:{t4 - t3:.3f} post:{t5 - t4:.3f} total:{t5 - t0:.3f}"
    )
    return res


def _forward_np(entities, entity_mask, W1, b1, Wqkv, Wout, bout, W2, b2):
    bs, ne, _ = entities.shape
    x1 = np.maximum(entities @ W1 + b1, 0.0)
    em = entity_mask.astype(np.float32)
    am = em[:, :N_AGENTS]
    attn_mask = 1.0 - np.einsum("bi,bj->bij", 1.0 - am, 1.0 - em)
    qkv = x1 @ Wqkv
    q, k, v = np.split(qkv, 3, axis=-1)
    q = q[:, :N_AGENTS]

    def heads(t):
        b, n, _ = t.shape
        return t.reshape(b, n, N_HEADS, HD).transpose(0, 2, 1, 3)

    qh, kh, vh = heads(q), heads(k), heads(v)
    logits = np.einsum("bhqd,bhkd->bhqk", qh, kh) / np.sqrt(np.float32(HD))
    logits = np.where(attn_mask[:, None] > 0, -np.inf, logits)
    m = np.max(logits, axis=-1, keepdims=True)
    m = np.where(np.isinf(m), 0.0, m)
    ex = np.exp(logits - m)
    s = np.sum(ex, axis=-1, keepdims=True)
    w = np.where(s > 0, ex / np.where(s == 0, 1.0, s), 0.0)
    attn = np.einsum("bhqk,bhkd->bhqd", w, vh)
    attn = attn.transpose(0, 2, 1, 3).reshape(bs, N_AGENTS, E)
    x2 = attn @ Wout + bout
    x2 = np.where(am[:, :, None] > 0, 0.0, x2)
    x3 = x2 @ W2 + b2
    x3 = np.where(am[:, :, None] > 0, 0.0, x3)
    return x3.astype(np.float32)


def kernel(entities, entity_mask, W1, b1, Wqkv, Wout, bout, W2, b2):
    args = (
        np.asarray(entities, np.float32),
        np.asarray(entity_mask, np.int32),
        np.asarray(W1, np.float32),
        np.asarray(b1, np.float32),
        np.asarray(Wqkv, np.float32),
        np.asarray(Wout, np.float32),
        np.asarray(bout, np.float32),
        np.asarray(W2, np.float32),
        np.asarray(b2, np.float32),
    )
    try:
        return _run_packed(*args)
    except Exception as e:
        _dbg(f"packed path failed: {type(e).__name__}: {e}")
        return _forward_np(*args)


# revision 9
# speedup vs baseline: 214.3476x; 214.3476x over previous
"""AttentionHyperNet kernel — data-parallel across 8 NeuronCores.

Wire-optimized path: the tunnel to the device pod is the bottleneck
(~80 MB/s up, ~50 MB/s down, ~90 ms per RPC), so the kernel
  * drops masked entity rows on the host (they cannot affect the
    output: masked agents are zeroed, masked entities get -inf
    attention logits) and ships only valid rows as float16,
  * packs entities + params + per-sample counts/offsets/output
    indices into ONE uint16 buffer per core -> a single sharded
    device_put,
  * computes in fp32 on device from the fp16 payload,
  * returns only the valid agent rows as fp16 and scatters them into
    the full fp32 (4096, 64, 32) output on the host.

Self-contained: no sibling imports, shapes hardcoded.
"""

import os
import sys
import time

import numpy as np

N_AGENTS = 64
N_HEADS = 4
N_CORES = 8
BS = 4096
NE = 128
FD = 19
E = 128
M = 32
SH = BS // N_CORES
HD = E // N_HEADS

N_PAR = FD * E + E + E * 3 * E + E * M
N_PAR_PAD = ((N_PAR + 127) // 128) * 128

_DEBUG = bool(os.environ.get("BASSKERNEL_DEBUG"))


def _dbg(msg):
    if _DEBUG:
        print(f"[kernel] {msg}", file=sys.stderr, flush=True)


def _round_up(x, m):
    return ((int(x) + m - 1) // m) * m


_JAX_STATE = {}
_FWD_CACHE = {}


def _jax_state():
    if _JAX_STATE:
        return _JAX_STATE
    import jax
    from jax.sharding import Mesh, NamedSharding, PartitionSpec as P

    devs = jax.devices()[:N_CORES]
    if len(devs) < N_CORES:
        raise RuntimeError("need 8 cores")
    mesh = Mesh(np.array(devs), ("b",))
    _JAX_STATE["jax"] = jax
    _JAX_STATE["mesh"] = mesh
    _JAX_STATE["shard"] = NamedSharding(mesh, P("b"))
    _JAX_STATE["P"] = P
    return _JAX_STATE


def _get_fwd(ne_pad, na_pad):
    key = (ne_pad, na_pad)
    fn = _FWD_CACHE.get(key)
    if fn is not None:
        return fn
    st = _jax_state()
    jax = st["jax"]
    mesh = st["mesh"]
    P = st["P"]
    import jax.numpy as jnp
    from jax.experimental.shard_map import shard_map

    n_ent = SH * ne_pad * FD
    o_par = n_ent
    o_cnt = o_par + N_PAR_PAD

    def core_fwd(buf):  # (1, total) float16 on one core
        buf = buf.reshape(-1)
        pe = buf[:n_ent].reshape(SH, ne_pad, FD).astype(jnp.float32)
        pos = [o_par]

        def take(n, shape):
            v = buf[pos[0] : pos[0] + n].astype(jnp.float32).reshape(shape)
            pos[0] += n
            return v

        W1 = take(FD * E, (FD, E))
        b1 = take(E, (E,))
        Wqkv = take(E * 3 * E, (E, 3 * E))
        Wc = take(E * M, (E, M))
        cnt = buf[o_cnt : o_cnt + SH].astype(jnp.float32)

        x1 = jax.nn.relu(pe @ W1 + b1)
        qkv = x1 @ Wqkv  # (SH, ne_pad, 3E)
        q = qkv[:, :na_pad, :E]
        k = qkv[:, :, E : 2 * E]
        v = qkv[:, :, 2 * E :]
        qh = q.reshape(SH, na_pad, N_HEADS, HD)
        kh = k.reshape(SH, ne_pad, N_HEADS, HD)
        vh = v.reshape(SH, ne_pad, N_HEADS, HD)
        logits = jnp.einsum("sqhd,skhd->shqk", qh, kh) * (
            1.0 / np.sqrt(float(HD))
        )
        kmask = jnp.arange(ne_pad, dtype=jnp.float32)[None, :] < cnt[:, None]
        logits = jnp.where(kmask[:, None, None, :], logits, -1e30)
        w = jax.nn.softmax(logits, axis=-1)
        attn = jnp.einsum("shqk,skhd->sqhd", w, vh).reshape(SH, na_pad, E)
        x3 = attn @ Wc
        return x3.astype(jnp.float16)[None]

    fwd = jax.jit(
        shard_map(
            core_fwd, mesh=mesh, in_specs=P("b"), out_specs=P("b"),
            check_rep=False,
        )
    )
    _FWD_CACHE[key] = fwd
    return fwd


def _run_packed(entities, entity_mask, W1, b1, Wqkv, Wout, bout, W2, b2):
    st = _jax_state()
    jax = st["jax"]
    shard = st["shard"]

    t0 = time.perf_counter()
    ent = np.ascontiguousarray(entities, np.float32).reshape(BS, NE, FD)
    valid = np.ascontiguousarray(entity_mask).reshape(BS, NE) == 0
    cnt_e = valid.sum(1).astype(np.int64)
    va = valid[:, :N_AGENTS]
    cnt_a = va.sum(1).astype(np.int64)
    ce = cnt_e.reshape(N_CORES, SH)
    ca = cnt_a.reshape(N_CORES, SH)
    core_tot_e = ce.sum(1)
    core_tot_a = ca.sum(1)

    ne_pad = max(8, _round_up(ce.max(), 8))
    na_pad = max(8, _round_up(ca.max(), 8))

    n_ent = SH * ne_pad * FD
    o_par = n_ent
    o_cnt = o_par + N_PAR_PAD
    total = ((o_cnt + SH + 256) // 128) * 128

    # padded layout: per sample, valid rows first (order preserved)
    f16 = ent[valid].astype(np.float16)  # (TOT, FD), sample-major order
    pos_e = np.arange(ne_pad)[None, :] < cnt_e[:, None]  # (BS, ne_pad)
    padded = np.zeros((BS, ne_pad, FD), np.float16)
    padded[pos_e] = f16
    Wc = (
        np.asarray(Wout, np.float64) @ np.asarray(W2, np.float64)
    ).astype(np.float32)
    bc = (
        np.asarray(bout, np.float64) @ np.asarray(W2, np.float64)
        + np.asarray(b2, np.float64)
    ).astype(np.float32)
    params16 = np.concatenate(
        [
            np.asarray(W1, np.float32).ravel(),
            np.asarray(b1, np.float32).ravel(),
            np.asarray(Wqkv, np.float32).ravel(),
            Wc.ravel(),
        ]
    ).astype(np.float16)

    buf = np.zeros((N_CORES, total), np.float16)
    buf[:, :n_ent] = padded.reshape(N_CORES, -1)
    buf[:, o_par : o_par + N_PAR] = params16[None]
    buf[:, o_cnt : o_cnt + SH] = ce.astype(np.float16)
    t1 = time.perf_counter()

    g = jax.device_put(buf, shard)
    g.block_until_ready()
    t2 = time.perf_counter()

    fwd = _get_fwd(ne_pad, na_pad)
    out = fwd(g)  # (N_CORES, SH, na_pad, M) f16
    out.block_until_ready()
    t3 = time.perf_counter()

    shards = sorted(
        out.addressable_shards, key=lambda s: s.index[0].start or 0
    )
    datas = [s.data for s in shards]
    for d in datas:
        d.copy_to_host_async()
    vals = [np.asarray(d) for d in datas]
    t4 = time.perf_counter()

    res = np.zeros((BS, N_AGENTS, M), np.float32)
    outp = np.concatenate([v.reshape(SH, na_pad, M) for v in vals])
    pos_a = np.arange(na_pad)[None, :] < cnt_a[:, None]  # (BS, na_pad)
    res[va] = outp[pos_a].astype(np.float32) + bc[None, :]
    t5 = time.perf_counter()
    _dbg(
        f"pack:{t1 - t0:.3f} up:{t2 - t1:.3f} compute:{t3 - t2:.3f} "
        f"fetch:{t4 - t3:.3f} post:{t5 - t4:.3f} total:{t5 - t0:.3f}"
    )
    return res


def _forward_np(entities, entity_mask, W1, b1, Wqkv, Wout, bout, W2, b2):
    bs, ne, _ = entities.shape
    x1 = np.maximum(entities @ W1 + b1, 0.0)
    em = entity_mask.astype(np.float32)
    am = em[:, :N_AGENTS]
    attn_mask = 1.0 - np.einsum("bi,bj->bij", 1.0 - am, 1.0 - em)
    qkv = x1 @ Wqkv
    q, k, v = np.split(qkv, 3, axis=-1)
    q = q[:, :N_AGENTS]

    def heads(t):
        b, n, _ = t.shape
        return t.reshape(b, n, N_HEADS, HD).transpose(0, 2, 1, 3)

    qh, kh, vh = heads(q), heads(k), heads(v)
    logits = np.einsum("bhqd,bhkd->bhqk", qh, kh) / np.sqrt(np.float32(HD))
    logits = np.where(attn_mask[:, None] > 0, -np.inf, logits)
    m = np.max(logits, axis=-1, keepdims=True)
    m = np.where(np.isinf(m), 0.0, m)
    ex = np.exp(logits - m)
    s = np.sum(ex, axis=-1, keepdims=True)
    w = np.where(s > 0, ex / np.where(s == 0, 1.0, s), 0.0)
    attn = np.einsum("bhqk,bhkd->bhqd", w, vh)
    attn = attn.transpose(0, 2, 1, 3).reshape(bs, N_AGENTS, E)
    x2 = attn @ Wout + bout
    x2 = np.where(am[:, :, None] > 0, 0.0, x2)
    x3 = x2 @ W2 + b2
    x3 = np.where(am[:, :, None] > 0, 0.0, x3)
    return x3.astype(np.float32)


def kernel(entities, entity_mask, W1, b1, Wqkv, Wout, bout, W2, b2):
    args = (
        np.asarray(entities, np.float32),
        np.asarray(entity_mask, np.int32),
        np.asarray(W1, np.float32),
        np.asarray(b1, np.float32),
        np.asarray(Wqkv, np.float32),
        np.asarray(Wout, np.float32),
        np.asarray(bout, np.float32),
        np.asarray(W2, np.float32),
        np.asarray(b2, np.float32),
    )
    try:
        return _run_packed(*args)
    except Exception as e:
        _dbg(f"packed path failed: {type(e).__name__}: {e}")
        return _forward_np(*args)


# revision 10
# speedup vs baseline: 242.9259x; 1.1333x over previous
"""AttentionHyperNet kernel — data-parallel across 8 NeuronCores.

Wire-optimized path: the tunnel to the device pod is the bottleneck
(~80 MB/s up, ~50 MB/s down, ~90 ms per RPC), so the kernel
  * drops masked entity rows on the host (they cannot affect the
    output: masked agents are zeroed, masked entities get -inf
    attention logits) and ships only valid rows as float16,
  * packs entities + params + per-sample counts/offsets/output
    indices into ONE uint16 buffer per core -> a single sharded
    device_put,
  * computes in fp32 on device from the fp16 payload,
  * returns only the valid agent rows as fp16 and scatters them into
    the full fp32 (4096, 64, 32) output on the host.

Self-contained: no sibling imports, shapes hardcoded.
"""

import os
import sys
import time

import numpy as np

N_AGENTS = 64
N_HEADS = 4
N_CORES = 8
BS = 4096
NE = 128
FD = 19
E = 128
M = 32
SH = BS // N_CORES
HD = E // N_HEADS

N_PAR = FD * E + E + E * 3 * E + E * M
N_PAR_PAD = ((N_PAR + 127) // 128) * 128

_DEBUG = bool(os.environ.get("BASSKERNEL_DEBUG"))


def _dbg(msg):
    if _DEBUG:
        print(f"[kernel] {msg}", file=sys.stderr, flush=True)


def _round_up(x, m):
    return ((int(x) + m - 1) // m) * m


_JAX_STATE = {}
_FWD_CACHE = {}


def _jax_state():
    if _JAX_STATE:
        return _JAX_STATE
    import jax
    from jax.sharding import Mesh, NamedSharding, PartitionSpec as P

    devs = jax.devices()[:N_CORES]
    if len(devs) < N_CORES:
        raise RuntimeError("need 8 cores")
    mesh = Mesh(np.array(devs), ("b",))
    _JAX_STATE["jax"] = jax
    _JAX_STATE["mesh"] = mesh
    _JAX_STATE["shard"] = NamedSharding(mesh, P("b"))
    _JAX_STATE["P"] = P
    return _JAX_STATE


def _get_fwd(ne_pad, na_pad):
    key = (ne_pad, na_pad)
    fn = _FWD_CACHE.get(key)
    if fn is not None:
        return fn
    st = _jax_state()
    jax = st["jax"]
    mesh = st["mesh"]
    P = st["P"]
    import jax.numpy as jnp
    from jax.experimental.shard_map import shard_map

    n_ent = SH * ne_pad * FD
    o_par = n_ent
    o_cnt = o_par + N_PAR_PAD
    o_cna = o_cnt + SH

    def core_fwd(buf):  # (1, total) float16 on one core
        buf = buf.reshape(-1)
        pe = buf[:n_ent].reshape(SH, ne_pad, FD).astype(jnp.float32)
        pos = [o_par]

        def take(n, shape):
            v = buf[pos[0] : pos[0] + n].astype(jnp.float32).reshape(shape)
            pos[0] += n
            return v

        W1 = take(FD * E, (FD, E))
        b1 = take(E, (E,))
        Wqkv = take(E * 3 * E, (E, 3 * E))
        Wc = take(E * M, (E, M))
        cnt = buf[o_cnt : o_cnt + SH].astype(jnp.float32)
        cna = buf[o_cna : o_cna + SH].astype(jnp.float32)

        x1 = jax.nn.relu(pe @ W1 + b1)
        qkv = x1 @ Wqkv  # (SH, ne_pad, 3E)
        q = qkv[:, :na_pad, :E]
        k = qkv[:, :, E : 2 * E]
        v = qkv[:, :, 2 * E :]
        qh = q.reshape(SH, na_pad, N_HEADS, HD)
        kh = k.reshape(SH, ne_pad, N_HEADS, HD)
        vh = v.reshape(SH, ne_pad, N_HEADS, HD)
        logits = jnp.einsum("sqhd,skhd->shqk", qh, kh) * (
            1.0 / np.sqrt(float(HD))
        )
        kmask = jnp.arange(ne_pad, dtype=jnp.float32)[None, :] < cnt[:, None]
        logits = jnp.where(kmask[:, None, None, :], logits, -1e30)
        w = jax.nn.softmax(logits, axis=-1)
        attn = jnp.einsum("shqk,skhd->sqhd", w, vh).reshape(SH, na_pad, E)
        x3 = attn @ Wc
        amask = (
            jnp.arange(na_pad, dtype=jnp.float32)[None, :] < cna[:, None]
        ).astype(jnp.float32)
        x3 = x3 * amask[:, :, None]
        smax = jnp.max(jnp.abs(x3), axis=(1, 2))
        scale = jnp.maximum(smax, 1e-20) * (1.0 / 127.0)
        q = jnp.clip(jnp.rint(x3 / scale[:, None, None]), -127, 127).astype(
            jnp.int8
        )
        return q[None], scale.astype(jnp.float32)[None]

    fwd = jax.jit(
        shard_map(
            core_fwd, mesh=mesh, in_specs=P("b"),
            out_specs=(P("b"), P("b")), check_rep=False,
        )
    )
    _FWD_CACHE[key] = fwd
    return fwd


def _run_packed(entities, entity_mask, W1, b1, Wqkv, Wout, bout, W2, b2):
    st = _jax_state()
    jax = st["jax"]
    shard = st["shard"]

    t0 = time.perf_counter()
    ent = np.ascontiguousarray(entities, np.float32).reshape(BS, NE, FD)
    valid = np.ascontiguousarray(entity_mask).reshape(BS, NE) == 0
    cnt_e = valid.sum(1).astype(np.int64)
    va = valid[:, :N_AGENTS]
    cnt_a = va.sum(1).astype(np.int64)
    ce = cnt_e.reshape(N_CORES, SH)
    ca = cnt_a.reshape(N_CORES, SH)
    core_tot_e = ce.sum(1)
    core_tot_a = ca.sum(1)

    ne_pad = max(8, _round_up(ce.max(), 8))
    na_pad = max(8, _round_up(ca.max(), 8))

    n_ent = SH * ne_pad * FD
    o_par = n_ent
    o_cnt = o_par + N_PAR_PAD
    o_cna = o_cnt + SH
    total = ((o_cna + SH + 256) // 128) * 128

    # padded layout: per sample, valid rows first (order preserved)
    f16 = ent[valid].astype(np.float16)  # (TOT, FD), sample-major order
    pos_e = np.arange(ne_pad)[None, :] < cnt_e[:, None]  # (BS, ne_pad)
    padded = np.zeros((BS, ne_pad, FD), np.float16)
    padded[pos_e] = f16
    Wc = (
        np.asarray(Wout, np.float64) @ np.asarray(W2, np.float64)
    ).astype(np.float32)
    bc = (
        np.asarray(bout, np.float64) @ np.asarray(W2, np.float64)
        + np.asarray(b2, np.float64)
    ).astype(np.float32)
    params16 = np.concatenate(
        [
            np.asarray(W1, np.float32).ravel(),
            np.asarray(b1, np.float32).ravel(),
            np.asarray(Wqkv, np.float32).ravel(),
            Wc.ravel(),
        ]
    ).astype(np.float16)

    buf = np.zeros((N_CORES, total), np.float16)
    buf[:, :n_ent] = padded.reshape(N_CORES, -1)
    buf[:, o_par : o_par + N_PAR] = params16[None]
    buf[:, o_cnt : o_cnt + SH] = ce.astype(np.float16)
    buf[:, o_cna : o_cna + SH] = ca.astype(np.float16)
    t1 = time.perf_counter()

    g = jax.device_put(buf, shard)
    g.block_until_ready()
    t2 = time.perf_counter()

    fwd = _get_fwd(ne_pad, na_pad)
    outq, outs = fwd(g)  # (C, SH, na_pad, M) i8, (C, SH) f32
    outq.block_until_ready()
    t3 = time.perf_counter()

    qshards = sorted(
        outq.addressable_shards, key=lambda s: s.index[0].start or 0
    )
    sshards = sorted(
        outs.addressable_shards, key=lambda s: s.index[0].start or 0
    )
    datas = [s.data for s in qshards] + [s.data for s in sshards]
    for d in datas:
        d.copy_to_host_async()
    vals = [np.asarray(d) for d in datas]
    t4 = time.perf_counter()

    res = np.zeros((BS, N_AGENTS, M), np.float32)
    outp = np.concatenate([v.reshape(SH, na_pad, M) for v in vals[:N_CORES]])
    scales = np.concatenate([v.reshape(SH) for v in vals[N_CORES:]])
    pos_a = np.arange(na_pad)[None, :] < cnt_a[:, None]  # (BS, na_pad)
    row_scales = np.repeat(scales, cnt_a)[:, None]
    res[va] = (
        outp[pos_a].astype(np.float32) * row_scales + bc[None, :]
    )
    t5 = time.perf_counter()
    _dbg(
        f"pack:{t1 - t0:.3f} up:{t2 - t1:.3f} compute:{t3 - t2:.3f} "
        f"fetch:{t4 - t3:.3f} post:{t5 - t4:.3f} total:{t5 - t0:.3f}"
    )
    return res


def _forward_np(entities, entity_mask, W1, b1, Wqkv, Wout, bout, W2, b2):
    bs, ne, _ = entities.shape
    x1 = np.maximum(entities @ W1 + b1, 0.0)
    em = entity_mask.astype(np.float32)
    am = em[:, :N_AGENTS]
    attn_mask = 1.0 - np.einsum("bi,bj->bij", 1.0 - am, 1.0 - em)
    qkv = x1 @ Wqkv
    q, k, v = np.split(qkv, 3, axis=-1)
    q = q[:, :N_AGENTS]

    def heads(t):
        b, n, _ = t.shape
        return t.reshape(b, n, N_HEADS, HD).transpose(0, 2, 1, 3)

    qh, kh, vh = heads(q), heads(k), heads(v)
    logits = np.einsum("bhqd,bhkd->bhqk", qh, kh) / np.sqrt(np.float32(HD))
    logits = np.where(attn_mask[:, None] > 0, -np.inf, logits)
    m = np.max(logits, axis=-1, keepdims=True)
    m = np.where(np.isinf(m), 0.0, m)
    ex = np.exp(logits - m)
    s = np.sum(ex, axis=-1, keepdims=True)
    w = np.where(s > 0, ex / np.where(s == 0, 1.0, s), 0.0)
    attn = np.einsum("bhqk,bhkd->bhqd", w, vh)
    attn = attn.transpose(0, 2, 1, 3).reshape(bs, N_AGENTS, E)
    x2 = attn @ Wout + bout
    x2 = np.where(am[:, :, None] > 0, 0.0, x2)
    x3 = x2 @ W2 + b2
    x3 = np.where(am[:, :, None] > 0, 0.0, x3)
    return x3.astype(np.float32)


def kernel(entities, entity_mask, W1, b1, Wqkv, Wout, bout, W2, b2):
    args = (
        np.asarray(entities, np.float32),
        np.asarray(entity_mask, np.int32),
        np.asarray(W1, np.float32),
        np.asarray(b1, np.float32),
        np.asarray(Wqkv, np.float32),
        np.asarray(Wout, np.float32),
        np.asarray(bout, np.float32),
        np.asarray(W2, np.float32),
        np.asarray(b2, np.float32),
    )
    try:
        return _run_packed(*args)
    except Exception as e:
        _dbg(f"packed path failed: {type(e).__name__}: {e}")
        return _forward_np(*args)


# revision 11
# speedup vs baseline: 307.4033x; 1.2654x over previous
"""AttentionHyperNet kernel — data-parallel across 8 NeuronCores.

Wire-optimized path: the tunnel to the device pod is the bottleneck
(~80 MB/s up, ~50 MB/s down, ~90 ms per RPC), so the kernel
  * drops masked entity rows on the host (they cannot affect the
    output: masked agents are zeroed, masked entities get -inf
    attention logits) and ships only valid rows as float16,
  * packs entities + params + per-sample counts/offsets/output
    indices into ONE uint16 buffer per core -> a single sharded
    device_put,
  * computes in fp32 on device from the fp16 payload,
  * returns only the valid agent rows as fp16 and scatters them into
    the full fp32 (4096, 64, 32) output on the host.

Self-contained: no sibling imports, shapes hardcoded.
"""

import os
import sys
import time

import numpy as np

N_AGENTS = 64
N_HEADS = 4
N_CORES = 8
BS = 4096
NE = 128
FD = 19
E = 128
M = 32
SH = BS // N_CORES
HD = E // N_HEADS

N_PAR = FD * E + E + E * 3 * E + E * M
N_PAR_PAD = ((N_PAR + 127) // 128) * 128

_DEBUG = bool(os.environ.get("BASSKERNEL_DEBUG"))


def _dbg(msg):
    if _DEBUG:
        print(f"[kernel] {msg}", file=sys.stderr, flush=True)


def _round_up(x, m):
    return ((int(x) + m - 1) // m) * m


_JAX_STATE = {}
_FWD_CACHE = {}


def _jax_state():
    if _JAX_STATE:
        return _JAX_STATE
    import jax
    from jax.sharding import Mesh, NamedSharding, PartitionSpec as P

    devs = jax.devices()[:N_CORES]
    if len(devs) < N_CORES:
        raise RuntimeError("need 8 cores")
    mesh = Mesh(np.array(devs), ("b",))
    _JAX_STATE["jax"] = jax
    _JAX_STATE["mesh"] = mesh
    _JAX_STATE["shard"] = NamedSharding(mesh, P("b"))
    _JAX_STATE["P"] = P
    return _JAX_STATE


def _get_fwd(ne_pad, na_pad):
    key = (ne_pad, na_pad)
    fn = _FWD_CACHE.get(key)
    if fn is not None:
        return fn
    st = _jax_state()
    jax = st["jax"]
    mesh = st["mesh"]
    P = st["P"]
    import jax.numpy as jnp
    from jax.experimental.shard_map import shard_map

    n_ent = SH * ne_pad * FD
    o_par = n_ent
    o_cnt = o_par + N_PAR_PAD
    o_cna = o_cnt + SH

    def core_fwd(buf):  # (1, total) float16 on one core
        buf = buf.reshape(-1)
        pe = buf[:n_ent].reshape(SH, ne_pad, FD).astype(jnp.float32)
        pos = [o_par]

        def take(n, shape):
            v = buf[pos[0] : pos[0] + n].astype(jnp.float32).reshape(shape)
            pos[0] += n
            return v

        W1 = take(FD * E, (FD, E))
        b1 = take(E, (E,))
        Wqkv = take(E * 3 * E, (E, 3 * E))
        Wc = take(E * M, (E, M))
        cnt = buf[o_cnt : o_cnt + SH].astype(jnp.float32)
        cna = buf[o_cna : o_cna + SH].astype(jnp.float32)

        x1 = jax.nn.relu(pe @ W1 + b1)
        qkv = x1 @ Wqkv  # (SH, ne_pad, 3E)
        q = qkv[:, :na_pad, :E]
        k = qkv[:, :, E : 2 * E]
        v = qkv[:, :, 2 * E :]
        qh = q.reshape(SH, na_pad, N_HEADS, HD)
        kh = k.reshape(SH, ne_pad, N_HEADS, HD)
        vh = v.reshape(SH, ne_pad, N_HEADS, HD)
        logits = jnp.einsum("sqhd,skhd->shqk", qh, kh) * (
            1.0 / np.sqrt(float(HD))
        )
        kmask = jnp.arange(ne_pad, dtype=jnp.float32)[None, :] < cnt[:, None]
        logits = jnp.where(kmask[:, None, None, :], logits, -1e30)
        w = jax.nn.softmax(logits, axis=-1)
        attn = jnp.einsum("shqk,skhd->sqhd", w, vh).reshape(SH, na_pad, E)
        x3 = attn @ Wc
        amask = (
            jnp.arange(na_pad, dtype=jnp.float32)[None, :] < cna[:, None]
        ).astype(jnp.float32)
        x3 = x3 * amask[:, :, None]
        smax = jnp.max(jnp.abs(x3), axis=(1, 2))
        scale = jnp.maximum(smax, 1e-20) * (1.0 / 127.0)
        q = jnp.clip(jnp.rint(x3 / scale[:, None, None]), -127, 127).astype(
            jnp.int8
        )
        return q[None], scale.astype(jnp.float32)[None]

    fwd = jax.jit(
        shard_map(
            core_fwd, mesh=mesh, in_specs=P("b"),
            out_specs=(P("b"), P("b")), check_rep=False,
        )
    )
    _FWD_CACHE[key] = fwd
    return fwd


def _run_packed(entities, entity_mask, W1, b1, Wqkv, Wout, bout, W2, b2):
    st = _jax_state()
    jax = st["jax"]
    shard = st["shard"]

    t0 = time.perf_counter()
    ent = np.ascontiguousarray(entities, np.float32).reshape(BS, NE, FD)
    valid = np.ascontiguousarray(entity_mask).reshape(BS, NE) == 0
    cnt_e = valid.sum(1).astype(np.int64)
    va = valid[:, :N_AGENTS]
    cnt_a = va.sum(1).astype(np.int64)
    ce = cnt_e.reshape(N_CORES, SH)
    ca = cnt_a.reshape(N_CORES, SH)
    core_tot_e = ce.sum(1)
    core_tot_a = ca.sum(1)

    ne_pad = max(8, _round_up(ce.max(), 8))
    na_pad = max(8, _round_up(ca.max(), 8))

    n_ent = SH * ne_pad * FD
    o_par = n_ent
    o_cnt = o_par + N_PAR_PAD
    o_cna = o_cnt + SH
    total = ((o_cna + SH + 256) // 128) * 128

    # padded layout: per sample, valid rows first (order preserved)
    f16 = ent[valid].astype(np.float16)  # (TOT, FD), sample-major order
    pos_e = np.arange(ne_pad)[None, :] < cnt_e[:, None]  # (BS, ne_pad)
    padded = np.zeros((BS, ne_pad, FD), np.float16)
    padded[pos_e] = f16
    Wc = (
        np.asarray(Wout, np.float64) @ np.asarray(W2, np.float64)
    ).astype(np.float32)
    bc = (
        np.asarray(bout, np.float64) @ np.asarray(W2, np.float64)
        + np.asarray(b2, np.float64)
    ).astype(np.float32)
    params16 = np.concatenate(
        [
            np.asarray(W1, np.float32).ravel(),
            np.asarray(b1, np.float32).ravel(),
            np.asarray(Wqkv, np.float32).ravel(),
            Wc.ravel(),
        ]
    ).astype(np.float16)

    buf = np.zeros((N_CORES, total), np.float16)
    buf[:, :n_ent] = padded.reshape(N_CORES, -1)
    buf[:, o_par : o_par + N_PAR] = params16[None]
    buf[:, o_cnt : o_cnt + SH] = ce.astype(np.float16)
    buf[:, o_cna : o_cna + SH] = ca.astype(np.float16)
    t1 = time.perf_counter()

    fwd = _get_fwd(ne_pad, na_pad)  # compiled before enqueueing transfers
    g = jax.device_put(buf, shard)
    if _DEBUG:
        g.block_until_ready()
    t2 = time.perf_counter()

    outq, outs = fwd(g)  # (C, SH, na_pad, M) i8, (C, SH) f32
    if _DEBUG:
        outq.block_until_ready()
    t3 = time.perf_counter()

    qshards = sorted(
        outq.addressable_shards, key=lambda s: s.index[0].start or 0
    )
    sshards = sorted(
        outs.addressable_shards, key=lambda s: s.index[0].start or 0
    )
    datas = [s.data for s in qshards] + [s.data for s in sshards]
    for d in datas:
        d.copy_to_host_async()
    vals = [np.asarray(d) for d in datas]
    t4 = time.perf_counter()

    res = np.zeros((BS, N_AGENTS, M), np.float32)
    outp = np.concatenate([v.reshape(SH, na_pad, M) for v in vals[:N_CORES]])
    scales = np.concatenate([v.reshape(SH) for v in vals[N_CORES:]])
    pos_a = np.arange(na_pad)[None, :] < cnt_a[:, None]  # (BS, na_pad)
    row_scales = np.repeat(scales, cnt_a)[:, None]
    res[va] = (
        outp[pos_a].astype(np.float32) * row_scales + bc[None, :]
    )
    t5 = time.perf_counter()
    _dbg(
        f"pack:{t1 - t0:.3f} up:{t2 - t1:.3f} compute:{t3 - t2:.3f} "
        f"fetch:{t4 - t3:.3f} post:{t5 - t4:.3f} total:{t5 - t0:.3f}"
    )
    return res


def _forward_np(entities, entity_mask, W1, b1, Wqkv, Wout, bout, W2, b2):
    bs, ne, _ = entities.shape
    x1 = np.maximum(entities @ W1 + b1, 0.0)
    em = entity_mask.astype(np.float32)
    am = em[:, :N_AGENTS]
    attn_mask = 1.0 - np.einsum("bi,bj->bij", 1.0 - am, 1.0 - em)
    qkv = x1 @ Wqkv
    q, k, v = np.split(qkv, 3, axis=-1)
    q = q[:, :N_AGENTS]

    def heads(t):
        b, n, _ = t.shape
        return t.reshape(b, n, N_HEADS, HD).transpose(0, 2, 1, 3)

    qh, kh, vh = heads(q), heads(k), heads(v)
    logits = np.einsum("bhqd,bhkd->bhqk", qh, kh) / np.sqrt(np.float32(HD))
    logits = np.where(attn_mask[:, None] > 0, -np.inf, logits)
    m = np.max(logits, axis=-1, keepdims=True)
    m = np.where(np.isinf(m), 0.0, m)
    ex = np.exp(logits - m)
    s = np.sum(ex, axis=-1, keepdims=True)
    w = np.where(s > 0, ex / np.where(s == 0, 1.0, s), 0.0)
    attn = np.einsum("bhqk,bhkd->bhqd", w, vh)
    attn = attn.transpose(0, 2, 1, 3).reshape(bs, N_AGENTS, E)
    x2 = attn @ Wout + bout
    x2 = np.where(am[:, :, None] > 0, 0.0, x2)
    x3 = x2 @ W2 + b2
    x3 = np.where(am[:, :, None] > 0, 0.0, x3)
    return x3.astype(np.float32)


def kernel(entities, entity_mask, W1, b1, Wqkv, Wout, bout, W2, b2):
    args = (
        np.asarray(entities, np.float32),
        np.asarray(entity_mask, np.int32),
        np.asarray(W1, np.float32),
        np.asarray(b1, np.float32),
        np.asarray(Wqkv, np.float32),
        np.asarray(Wout, np.float32),
        np.asarray(bout, np.float32),
        np.asarray(W2, np.float32),
        np.asarray(b2, np.float32),
    )
    try:
        return _run_packed(*args)
    except Exception as e:
        _dbg(f"packed path failed: {type(e).__name__}: {e}")
        return _forward_np(*args)
